# revision 7
# baseline (speedup 1.0000x reference)
"""Trilinear interpolation (grid_sample) on 8 TRN2 NeuronCores.

Strategy:
- Host: channel-last + edge-pad the (16,128,128,128) volume, then build an
  8-corner-expanded row table: row(x,y,z) = all 8 corners x 16 ch = 512B.
  Shard x into 8 slabs of 16 planes (one per core, 128MB each).
- Host: bin the 1M points by x-window (2 planes = 32768 rows, fits int16
  indexing) -> 64 bins, 8 per core; pad each bin to a chunk multiple.
- Device (per core): DVE computes floor/frac/corner-weights + int16 row
  indices; one 512B dma_gather per point from the core's slab; DVE
  broadcast-mul by the 8 corner weights and tree-reduces; DMA out.
- Host: inverse-permute to the full (16, 1000000) output.
"""
import numpy as np

import concourse.bass as bass
import concourse.tile as tile
from concourse import bacc, mybir
from concourse import bass_utils

P = 128
C = 16              # channels
D = 128             # grid size per dim
CH = 8192           # points per gather chunk
ROW = 128           # f32 per expanded row (8 corners * 16 ch)
WINDOW = 2 * D * D  # rows per gather window (2 x-planes) = 32768
NCORES = 8
XPL = D // NCORES   # x-planes per core = 16
BINS = NCORES * XPL // 2  # 64 global windows, 8 per core

_cache = {}
RUN_CORES = 8   # override <8 for debugging: only first k cores run on HW


def _build(nch, cpb, reg_counts):
    """Build the SPMD Bass program. nch = chunks per core, cpb = chunks per
    bin, reg_counts[c][k] = valid idx count for core c chunk k (only used to
    skip fully-empty chunks; gathers always use num_idxs_reg=CH when any)."""
    U = nch * CH // P          # planeA cols per partition
    M = U                      # planeB cols per partition (= total_tblcols/8)
    TBL = nch * CH // 16       # table cols (replicated layout)
    f32, i32, i16 = mybir.dt.float32, mybir.dt.int32, mybir.dt.int16

    nc = bacc.Bacc("TRN2", target_bir_lowering=False, debug=False,
                   num_devices=RUN_CORES)
    vol = nc.dram_tensor("vol", [XPL * D * D, ROW], f32, kind="ExternalInput")
    pax = nc.dram_tensor("pax", [P, U], f32, kind="ExternalInput")
    pay = nc.dram_tensor("pay", [P, U], f32, kind="ExternalInput")
    paz = nc.dram_tensor("paz", [P, U], f32, kind="ExternalInput")
    pbx = nc.dram_tensor("pbx", [P, M], f32, kind="ExternalInput")
    pby = nc.dram_tensor("pby", [P, M], f32, kind="ExternalInput")
    pbz = nc.dram_tensor("pbz", [P, M], f32, kind="ExternalInput")
    xbb = nc.dram_tensor("xbb", [P, M], f32, kind="ExternalInput")
    out = nc.dram_tensor("out", [P, U * C], f32, kind="ExternalOutput")

    gt = mybir.AluOpType.is_gt
    anybin = [any(reg_counts[c][k] for c in range(NCORES))
              for k in range(nch)]

    with tile.TileContext(nc) as tc:
        with tc.tile_pool(name="persist", bufs=1) as pp, \
             tc.tile_pool(name="dram", bufs=1, space="DRAM") as dp:
            table = pp.tile([P, TBL], i16)
            w8 = pp.tile([P, U * 8], f32)

            # ---------- idx path (planeB layout) ----------
            with tc.tile_pool(name="prepB", bufs=1) as pb:
                def floor_of(src_dram, name):
                    cc = pb.tile([P, M], f32, tag=f"c{name}")
                    nc.sync.dma_start(cc[:], src_dram.ap())
                    nc.vector.tensor_scalar(cc[:], cc[:], 1.0, 63.5,
                                            mybir.AluOpType.add,
                                            mybir.AluOpType.mult)
                    fi = pb.tile([P, M], i32, tag=f"fi{name}")
                    nc.vector.tensor_copy(fi[:], cc[:])
                    ff = pb.tile([P, M], f32, tag=f"ff{name}")
                    nc.vector.tensor_copy(ff[:], fi[:])
                    adj = pb.tile([P, M], f32, tag=f"adj{name}")
                    nc.vector.tensor_tensor(adj[:], ff[:], cc[:], gt)
                    nc.vector.tensor_sub(ff[:], ff[:], adj[:])
                    return ff

                fxB = floor_of(pbx, "x")
                xb = pb.tile([P, M], f32)
                nc.sync.dma_start(xb[:], xbb.ap())
                nc.vector.tensor_sub(fxB[:], fxB[:], xb[:])   # parity
                nc.vector.tensor_scalar_max(fxB[:], fxB[:], 0.0)
                nc.vector.tensor_scalar_min(fxB[:], fxB[:], 1.0)
                fyB = floor_of(pby, "y")
                fzB = floor_of(pbz, "z")
                idxf = pb.tile([P, M], f32)
                nc.vector.tensor_scalar_mul(idxf[:], fxB[:], float(WINDOW // 2))
                nc.vector.tensor_scalar_mul(fyB[:], fyB[:], float(D))
                nc.vector.tensor_add(idxf[:], idxf[:], fyB[:])
                nc.vector.tensor_add(idxf[:], idxf[:], fzB[:])
                idxi = pb.tile([P, M], i32)
                nc.vector.tensor_copy(idxi[:], idxf[:])
                idx16 = pb.tile([P, M], i16)
                nc.vector.tensor_copy(idx16[:], idxi[:])

                scratch = dp.tile([P, M], i16)
                nc.sync.dma_start(scratch[:], idx16[:])
                s = scratch[:]
                rd = bass.AP(s.tensor, s.offset, [[M, 16], [16 * M, 8], [1, M]])
                for j in range(8):
                    dst = table[:][16 * j:16 * (j + 1), :]
                    dst3 = bass.AP(dst.tensor, dst.offset,
                                   [dst.ap[0], [M, 8], [1, M]])
                    nc.sync.dma_start(dst3, rd)

            # ---------- weights path (planeA layout) ----------
            with tc.tile_pool(name="prepA", bufs=1) as pa:
                def frac_of(src_dram, name):
                    cc = pa.tile([P, U], f32, tag=f"c{name}")
                    nc.sync.dma_start(cc[:], src_dram.ap())
                    nc.vector.tensor_scalar(cc[:], cc[:], 1.0, 63.5,
                                            mybir.AluOpType.add,
                                            mybir.AluOpType.mult)
                    fi = pa.tile([P, U], i32, tag=f"fi{name}")
                    nc.vector.tensor_copy(fi[:], cc[:])
                    ff = pa.tile([P, U], f32, tag=f"ff{name}")
                    nc.vector.tensor_copy(ff[:], fi[:])
                    adj = pa.tile([P, U], f32, tag=f"adj{name}")
                    nc.vector.tensor_tensor(adj[:], ff[:], cc[:], gt)
                    nc.vector.tensor_sub(ff[:], ff[:], adj[:])
                    nc.vector.tensor_sub(cc[:], cc[:], ff[:])  # frac
                    return cc

                frx = frac_of(pax, "x")
                fry = frac_of(pay, "y")
                frz = frac_of(paz, "z")

                def wpair(fr, name):
                    w = pa.tile([P, U * 2], f32, tag=f"w{name}")
                    wv = w[:].rearrange("p (u two) -> p u two", two=2)
                    nc.vector.tensor_scalar(wv[:, :, 0], fr[:], -1.0, 1.0,
                                            mybir.AluOpType.mult,
                                            mybir.AluOpType.add)
                    nc.vector.tensor_copy(wv[:, :, 1], fr[:])
                    return w

                WX, WY, WZ = wpair(frx, "x"), wpair(fry, "y"), wpair(frz, "z")
                wyz = pa.tile([P, U * 4], f32)
                ay = WY[:]; az = WZ[:]
                nc.vector.tensor_mul(
                    bass.AP(wyz[:].tensor, wyz[:].offset,
                            [wyz[:].ap[0], [4, U], [2, 2], [1, 2]]),
                    bass.AP(ay.tensor, ay.offset,
                            [ay.ap[0], [2, U], [1, 2], [0, 2]]),
                    bass.AP(az.tensor, az.offset,
                            [az.ap[0], [2, U], [0, 2], [1, 2]]))
                ax = WX[:]; ayz = wyz[:]
                nc.vector.tensor_mul(
                    bass.AP(w8[:].tensor, w8[:].offset,
                            [w8[:].ap[0], [8, U], [4, 2], [1, 4]]),
                    bass.AP(ax.tensor, ax.offset,
                            [ax.ap[0], [2, U], [1, 2], [0, 4]]),
                    bass.AP(ayz.tensor, ayz.offset,
                            [ayz.ap[0], [4, U], [0, 2], [1, 4]]))

            # ---------- main loop ----------
            with tc.tile_pool(name="g", bufs=2) as gp, \
                 tc.tile_pool(name="red", bufs=1) as rp, \
                 tc.tile_pool(name="o", bufs=2) as op_:
                for k in range(nch):
                    g = gp.tile([P, (CH // P) * ROW], f32, tag="g")
                    if anybin[k]:
                        b = k // cpb
                        g3 = g[:].rearrange("p (s e) -> p s e", e=ROW)
                        win = vol.ap()[b * WINDOW:(b + 1) * WINDOW, :]
                        nc.gpsimd.dma_gather(
                            out_ap=g3, in_ap=win,
                            idxs_ap=table[:, k * (CH // 16):(k + 1) * (CH // 16)],
                            num_idxs=CH, num_idxs_reg=CH, elem_size=ROW,
                            single_packet=False)
                    else:
                        nc.vector.memzero(g[:])
                    def view(ap, dims):
                        return bass.AP(ap.tensor, ap.offset, [ap.ap[0]] + dims)

                    S = CH // P
                    gv4 = view(g[:], [[128, S], [16, 8], [1, 16]])
                    w8v = view(w8[:, k * S * 8:(k + 1) * S * 8],
                               [[8, S], [1, 8], [0, 16]])
                    nc.vector.tensor_mul(gv4, gv4, w8v)
                    s1 = rp.tile([P, S * 64], f32, tag="s1")
                    nc.vector.tensor_add(
                        view(s1[:], [[64, S], [1, 64]]),
                        view(g[:], [[128, S], [1, 64]]),
                        view(g[:, 64:], [[128, S], [1, 64]]))
                    s2 = rp.tile([P, S * 32], f32, tag="s2")
                    nc.vector.tensor_add(
                        view(s2[:], [[32, S], [1, 32]]),
                        view(s1[:], [[64, S], [1, 32]]),
                        view(s1[:, 32:], [[64, S], [1, 32]]))
                    ot = op_.tile([P, S * C], f32, tag="ot")
                    nc.vector.tensor_add(
                        view(ot[:], [[16, S], [1, 16]]),
                        view(s2[:], [[32, S], [1, 16]]),
                        view(s2[:, 16:], [[32, S], [1, 16]]))
                    nc.sync.dma_start(
                        out.ap()[:, k * (CH // P) * C:(k + 1) * (CH // P) * C],
                        ot[:])
    nc.compile()
    return nc


def kernel(input, coords):
    input = np.asarray(input, dtype=np.float32)
    coords = np.asarray(coords, dtype=np.float32)
    N = coords.shape[0]

    # exact same f32 math as the device for binning
    cx = (coords[:, 0] + np.float32(1.0)) * np.float32(63.5)
    fx = np.floor(cx).astype(np.int64)
    np.clip(fx, 0, D - 2, out=fx)
    wglob = fx >> 1                       # 0..63
    core_of = (wglob // (XPL // 2)).astype(np.int64)   # 8 windows per core
    bin_of = (wglob % (XPL // 2)).astype(np.int64)

    order = np.lexsort((np.arange(N), bin_of + 8 * core_of))
    key = (bin_of + 8 * core_of)[order]
    counts = np.bincount(key, minlength=64)
    capb = max(CH, int(np.ceil(counts.max() / CH)) * CH)
    cpb = capb // CH
    nch = 8 * cpb
    U = nch * CH // P
    M = U

    # per-(core,bin) valid counts per chunk
    reg_counts = [[0] * nch for _ in range(NCORES)]
    for c in range(NCORES):
        for b in range(8):
            n = int(counts[c * 8 + b])
            for kk in range(cpb):
                reg_counts[c][b * cpb + kk] = min(max(n - kk * CH, 0), CH)

    # ---------- expanded volume slabs ----------
    Vt = np.ascontiguousarray(input.transpose(1, 2, 3, 0))   # (x,y,z,ch)
    Vp = np.pad(Vt, ((0, 1), (0, 1), (0, 1), (0, 0)), mode="edge")
    vols = []
    for c in range(NCORES):
        E = np.empty((XPL, D, D, 8, C), np.float32)
        for dx in range(2):
            for dy in range(2):
                for dz in range(2):
                    j = dx * 4 + dy * 2 + dz
                    E[:, :, :, j, :] = Vp[16 * c + dx:16 * c + XPL + dx,
                                          dy:D + dy, dz:D + dz, :]
        vols.append(E.reshape(XPL * D * D, ROW))

    # ---------- per-core point layouts ----------
    i_all = np.empty(64 * capb, np.int64)       # padded slot -> orig idx (-1 pad)
    i_all.fill(-1)
    starts = np.zeros(65, np.int64)
    np.cumsum(counts, out=starts[1:])
    for gb in range(64):
        n = int(counts[gb])
        i_all[gb * capb:gb * capb + n] = order[starts[gb]:starts[gb] + n]

    in_maps = []
    core_meta = []
    for c in range(NCORES):
        ids = i_all[c * 8 * capb:(c + 1) * 8 * capb]       # [8*capb]
        valid = ids >= 0
        # pad coords: center of the bin's first plane, y=z=center
        padu = np.empty((ids.size, 3), np.float32)
        binidx = np.arange(ids.size) // capb
        padu[:, 0] = (2 * (8 * c + binidx) + 0.5) / np.float32(63.5) - 1.0
        padu[:, 1:] = 0.0
        cc = padu.copy()
        cc[valid] = coords[ids[valid]]

        # planeA: point slot i (within core) -> chunk k=i//CH, r=i%CH,
        #   p=r%128, u = k*64 + r//128
        i_lin = np.arange(ids.size)
        kk = i_lin // CH
        r = i_lin % CH
        pa_p = r % P
        pa_u = kk * (CH // P) + r // P
        planeA = np.empty((3, P, U), np.float32)
        planeA[:, pa_p, pa_u] = cc.T
        # planeB: q=r%16, scol = k*512 + r//16; j=scol//M, colB=scol%M
        q = r % 16
        scol = kk * (CH // 16) + r // 16
        jj = scol // M
        colB = scol % M
        planeB = np.empty((3, P, M), np.float32)
        planeB[:, 16 * jj + q, colB] = cc.T
        xbb = np.empty((P, M), np.float32)
        xbb[16 * jj + q, colB] = (2.0 * (8 * c + binidx)).astype(np.float32)

        in_maps.append({
            "vol": vols[c],
            "pax": np.ascontiguousarray(planeA[0]),
            "pay": np.ascontiguousarray(planeA[1]),
            "paz": np.ascontiguousarray(planeA[2]),
            "pbx": np.ascontiguousarray(planeB[0]),
            "pby": np.ascontiguousarray(planeB[1]),
            "pbz": np.ascontiguousarray(planeB[2]),
            "xbb": xbb,
        })
        core_meta.append((ids, valid, pa_p, pa_u))

    key_cfg = (nch, cpb, tuple(tuple(rc) for rc in reg_counts))
    if key_cfg not in _cache:
        _cache.clear()
        _cache[key_cfg] = _build(nch, cpb, reg_counts)
    nc = _cache[key_cfg]

    import time as _time
    _t0 = _time.perf_counter()
    res = bass_utils.run_bass_kernel_spmd(
        nc, in_maps[:RUN_CORES], core_ids=list(range(RUN_CORES)))
    global LAST_EXEC_S
    LAST_EXEC_S = _time.perf_counter() - _t0
    if RUN_CORES < NCORES:
        z = np.zeros_like(res.results[0]["out"])
        res.results = list(res.results) + [
            {"out": z} for _ in range(NCORES - RUN_CORES)]

    outf = np.empty((C, N), np.float32)
    for c in range(NCORES):
        ids, valid, pa_p, pa_u = core_meta[c]
        vals = res.results[c]["out"].reshape(P, U, C)
        outf[:, ids[valid]] = vals[pa_p[valid], pa_u[valid], :].T
    return outf
